# revision 4
# baseline (speedup 1.0000x reference)
"""Trainium2 Bass kernel for fused Luong 'general' attention.

Reference computation (jax):
    energy[s,b,k]       = sum_h enc[s,b,h] * W[k,h] + b_attn[k]
    attn_energies[b,s]  = sum_k hidden[0,b,k] * energy[s,b,k]
    out                 = softmax(attn_energies, axis=1)[:, None, :]   # [B,1,S]

Key algebra: attn_energies[b,s] = sum_h (sum_k hidden[b,k] W[k,h]) enc[s,b,h]
                                  + sum_k hidden[b,k] b_attn[k]
The b_attn term is constant in s, so it cancels exactly under softmax over s.
With v = hidden[0] @ W  ([B,H]), the kernel is just

    out[b, 0, s] = softmax_s( v[b,:] . enc[s,b,:] )

which is DMA-bound (enc is 256 MB); the big [S,B,H]x[H,H] matmul of the
reference never needs to be materialized.

Distribution: data-parallel over batch B=32 across 8 cores (4 each). Each
core's enc slice is re-laid-out host-side to [b, h, s] so that the h
(contraction) dim sits on SBUF partitions: the dot product then runs as
TensorE matvecs (stationary = vT[:, b] column, moving = enc tile), with
e[1, S] accumulated in PSUM across h-chunks. Softmax per b is local to the
core (no collectives): DVE reduce-max (negated) -> ScalarE Exp with bias =
-max and fused accum_out sum -> DVE reciprocal -> tensor_scalar multiply.
"""

import sys

for _p in (
    "/root/.axon_site",
    "/root/.axon_site/_ro/trn_rl_repo",
    "/root/.axon_site/_ro/pypackages",
):
    if _p not in sys.path:
        sys.path.append(_p)

import numpy as np

import concourse.bass as bass
import concourse.tile as tile
from concourse import bacc, mybir
from concourse.bass_utils import run_bass_kernel_spmd

S, B, H = 2048, 32, 1024
N_CORES = 8
B_LOC = B // N_CORES  # batches per core

F32 = mybir.dt.float32
P = 128  # SBUF partitions
SCHUNK = 512  # max fp32 moving free dim per matmul (one PSUM bank)


def build_program(b_loc=B_LOC, h=H, s=S, n_devices=N_CORES, enc_bufs=12):
    """Emit the per-core SPMD Tile program.

    Inputs (per core):
      enc  [b_loc, h, s]  f32 -- encoder slice, (b, h, s) layout
      hidT [h, b_loc]     f32 -- hidden slice, transposed
      w    [h, h]         f32 -- full W_attn (w[k, j] multiplies hidden k -> v j)
    Output:
      out  [b_loc, s]     f32 -- softmax over s of the attention energies
    """
    assert h % P == 0 and s % SCHUNK == 0
    hc_n = h // P  # h-chunks of 128 (contraction tiles)
    sc_n = s // SCHUNK  # s-chunks of 512 (PSUM banks)

    # Bacc (not raw Bass): its compile() legalizes multi-sem-wait matmuls
    # (move_matmul_waits_to_ldweights + generate_event_semaphores) — walrus
    # rejects a Matmult carrying >1 sync wait otherwise.
    nc = bacc.Bacc(
        "TRN2", target_bir_lowering=False, debug=False, num_devices=n_devices
    )
    enc = nc.dram_tensor("enc", [b_loc, h, s], F32, kind="ExternalInput").ap()
    hidT = nc.dram_tensor("hidT", [h, b_loc], F32, kind="ExternalInput").ap()
    w = nc.dram_tensor("w", [h, h], F32, kind="ExternalInput").ap()
    out = nc.dram_tensor("out", [b_loc, s], F32, kind="ExternalOutput").ap()

    with tile.TileContext(nc) as tc:
        with (
            tc.tile_pool(name="wts", bufs=hc_n) as wpool,
            tc.tile_pool(name="consts", bufs=1) as consts,
            tc.tile_pool(name="encp", bufs=enc_bufs) as encp,
            tc.tile_pool(name="psum", bufs=2, space="PSUM") as psp,
            tc.tile_pool(name="small", bufs=4) as small,
        ):
            # hidT [h, b_loc] -> SBUF [128, hc_n, b_loc] (k-chunk c holds rows
            # c*128 + p)
            hidT_sb = consts.tile([P, hc_n, b_loc], F32)
            nc.sync.dma_start(
                out=hidT_sb, in_=hidT.rearrange("(c p) b -> p c b", p=P)
            )

            # W k-chunk tiles [128, h]; all hc_n live at once (each vT chunk
            # contracts over every k-chunk)
            w_tiles = []
            for kc in range(hc_n):
                wt = wpool.tile([P, h], F32, tag="w")
                nc.sync.dma_start(out=wt, in_=w[kc * P : (kc + 1) * P, :])
                w_tiles.append(wt)

            # vT[hc][p, b] = v[b, hc*128 + p] = sum_k hidden[b,k] W[k, hc*128+p]
            # column layout: vT[:, hc*b_loc + b]
            vT = consts.tile([P, hc_n * b_loc], F32)
            for hcc in range(hc_n):
                pv = psp.tile([P, b_loc], F32, tag="ps")
                for kc in range(hc_n):
                    nc.tensor.matmul(
                        pv,
                        w_tiles[kc][:, hcc * P : (hcc + 1) * P],
                        hidT_sb[:, kc, :],
                        start=(kc == 0),
                        stop=(kc == hc_n - 1),
                    )
                nc.vector.tensor_copy(vT[:, hcc * b_loc : (hcc + 1) * b_loc], pv)

            for bl in range(b_loc):
                etiles = []
                for hcc in range(hc_n):
                    t = encp.tile([P, s], F32, tag="enc")
                    nc.sync.dma_start(
                        out=t, in_=enc[bl, hcc * P : (hcc + 1) * P, :]
                    )
                    etiles.append(t)

                # e[1, s] accumulated over h-chunks; each matmul writes one
                # 512-wide PSUM bank
                eps = psp.tile([1, s], F32, tag="ps")
                for hcc in range(hc_n):
                    lhs = vT[:, hcc * b_loc + bl : hcc * b_loc + bl + 1]
                    for sc in range(sc_n):
                        nc.tensor.matmul(
                            eps[0:1, sc * SCHUNK : (sc + 1) * SCHUNK],
                            lhs,
                            etiles[hcc][:, sc * SCHUNK : (sc + 1) * SCHUNK],
                            start=(hcc == 0),
                            stop=(hcc == hc_n - 1),
                        )

                # softmax over s (single partition lane)
                negmax = small.tile([1, 1], F32, tag="negmax")
                nc.vector.tensor_reduce(
                    negmax,
                    eps[0:1, :],
                    axis=mybir.AxisListType.X,
                    op=mybir.AluOpType.max,
                    negate=True,
                )
                psb = small.tile([1, s], F32, tag="p")
                ssum = small.tile([1, 1], F32, tag="ssum")
                nc.scalar.activation(
                    psb,
                    eps[0:1, :],
                    mybir.ActivationFunctionType.Exp,
                    bias=negmax,
                    scale=1.0,
                    accum_out=ssum,
                )
                rinv = small.tile([1, 1], F32, tag="rinv")
                nc.vector.reciprocal(rinv, ssum)
                osb = small.tile([1, s], F32, tag="o")
                nc.vector.tensor_scalar_mul(osb, psb, rinv)
                nc.sync.dma_start(out=out[bl : bl + 1, :], in_=osb)

    nc.compile()
    return nc


def _make_in_maps(hidden, encoder_outputs, W_attn):
    hidden = np.ascontiguousarray(np.asarray(hidden, dtype=np.float32))
    enc = np.asarray(encoder_outputs, dtype=np.float32)
    W = np.ascontiguousarray(np.asarray(W_attn, dtype=np.float32))

    # [S, B, H] -> [B, H, S] relayout (pure data movement, part of sharding)
    encT = np.ascontiguousarray(enc.transpose(1, 2, 0))
    hidT_full = np.ascontiguousarray(hidden[0].T)  # [H, B]

    in_maps = []
    for i in range(N_CORES):
        lo, hi = i * B_LOC, (i + 1) * B_LOC
        in_maps.append(
            {
                "enc": np.ascontiguousarray(encT[lo:hi]),
                "hidT": np.ascontiguousarray(hidT_full[:, lo:hi]),
                "w": W,
            }
        )
    return in_maps


def run_spmd(hidden, encoder_outputs, W_attn, b_attn=None, trace=False):
    """Run on all 8 cores; returns (out [B,1,S], BassKernelResults)."""
    in_maps = _make_in_maps(hidden, encoder_outputs, W_attn)
    nc = build_program()
    res = run_bass_kernel_spmd(nc, in_maps, list(range(N_CORES)), trace=trace)
    out = np.concatenate([r["out"] for r in res.results], axis=0)  # [B, S]
    return np.ascontiguousarray(out[:, None, :].astype(np.float32)), res


def kernel(hidden, encoder_outputs, W_attn, b_attn):
    # b_attn contributes a per-b constant to the energies; softmax over s is
    # invariant to it, so it is (exactly) unused.
    out, _ = run_spmd(hidden, encoder_outputs, W_attn, b_attn)
    return out


# revision 8
# speedup vs baseline: 1.1391x; 1.1391x over previous
"""Trainium2 Bass kernel for fused Luong 'general' attention.

Reference computation (jax):
    energy[s,b,k]       = sum_h enc[s,b,h] * W[k,h] + b_attn[k]
    attn_energies[b,s]  = sum_k hidden[0,b,k] * energy[s,b,k]
    out                 = softmax(attn_energies, axis=1)[:, None, :]   # [B,1,S]

Key algebra: attn_energies[b,s] = sum_h (sum_k hidden[b,k] W[k,h]) enc[s,b,h]
                                  + sum_k hidden[b,k] b_attn[k]
The b_attn term is constant in s, so it cancels exactly under softmax over s.
With v = hidden[0] @ W  ([B,H]), the kernel is just

    out[b, 0, s] = softmax_s( v[b,:] . enc[s,b,:] )

which is DMA-bound (enc is 256 MB); the big [S,B,H]x[H,H] matmul of the
reference never needs to be materialized.

Distribution: data-parallel over batch B=32 across 8 cores (4 each). Each
core's enc slice is re-laid-out host-side to [b, h, s] so that the h
(contraction) dim sits on SBUF partitions, and split into a bf16 hi/lo pair
(enc = ehi + elo exactly to ~16 mantissa bits; same total DMA bytes as f32).
The dot product then runs as TensorE matvecs at bf16 rate (fp32 matmul is 4
cycles/column on trn2 — LOW/HIGH passes x 2 cycles; bf16 is 1): with
v = vhi + vlo split the same way,

    e  =  vhi.ehi + vhi.elo + vlo.ehi     (dropped vlo.elo term ~2^-18 rel)

i.e. 3 bf16 matmuls per tile instead of one 4x-cost fp32 matmul. e[1, S] is
accumulated in PSUM across h-chunks and terms. v itself is computed in f32
with W as the *moving* operand (stationary W would pay 64 fp32 [128,128]
weight loads), then transposed 4x128 -> 128x4 on the PE via identity matmul.
Softmax per b is local to the core (no collectives): DVE reduce-max
(negated) -> ScalarE Exp with bias = -max and fused accum_out sum -> DVE
reciprocal -> tensor_scalar multiply.
"""

import sys

for _p in (
    "/root/.axon_site",
    "/root/.axon_site/_ro/trn_rl_repo",
    "/root/.axon_site/_ro/pypackages",
):
    if _p not in sys.path:
        sys.path.append(_p)

import ml_dtypes
import numpy as np

import concourse.bass as bass
import concourse.tile as tile
from concourse import bacc, mybir
from concourse.bass_utils import run_bass_kernel_spmd
from concourse.masks import make_identity

S, B, H = 2048, 32, 1024
N_CORES = 8
B_LOC = B // N_CORES  # batches per core

F32 = mybir.dt.float32
BF16 = mybir.dt.bfloat16
P = 128  # SBUF partitions
SCHUNK = 512  # PSUM-bank-sized matmul free dim
BF16_NP = ml_dtypes.bfloat16


def build_program(b_loc=B_LOC, h=H, s=S, n_devices=N_CORES, enc_bufs=2):
    """Emit the per-core SPMD Tile program.

    Inputs (per core):
      ehi, elo [b_loc, h, s] bf16 -- encoder slice, (b, h, s) layout, hi/lo split
      hidT     [h, b_loc]    f32  -- hidden slice, transposed
      w        [h, h]        f32  -- full W_attn (w[k, j]: hidden k -> v j)
    Output:
      out      [b_loc, s]    f32  -- softmax over s of the attention energies
    """
    assert h % P == 0 and s % SCHUNK == 0
    hc_n = h // P  # h-chunks of 128 (contraction tiles)
    sc_n = s // SCHUNK  # s-chunks of 512 (PSUM banks)
    hc_half = hc_n // 2 if hc_n % 2 == 0 else hc_n  # enc DMA split granularity

    # Bacc (not raw Bass): its compile() legalizes multi-sem-wait matmuls
    # (move_matmul_waits_to_ldweights + generate_event_semaphores) — walrus
    # rejects a Matmult carrying >1 sync wait otherwise.
    nc = bacc.Bacc(
        "TRN2", target_bir_lowering=False, debug=False, num_devices=n_devices
    )
    ehi = nc.dram_tensor("ehi", [b_loc, h, s], BF16, kind="ExternalInput").ap()
    elo = nc.dram_tensor("elo", [b_loc, h, s], BF16, kind="ExternalInput").ap()
    hidT = nc.dram_tensor("hidT", [h, b_loc], F32, kind="ExternalInput").ap()
    w = nc.dram_tensor("w", [h, h], F32, kind="ExternalInput").ap()
    out = nc.dram_tensor("out", [b_loc, s], F32, kind="ExternalOutput").ap()

    with tile.TileContext(nc) as tc:
        with (
            tc.tile_pool(name="consts", bufs=1) as consts,
            tc.tile_pool(name="encp", bufs=enc_bufs) as encp,
            tc.tile_pool(name="psum", bufs=2, space="PSUM") as psp,
            tc.tile_pool(name="small", bufs=2) as small,
        ):
            # ---- phase 1: vT = (hidden @ W)^T, f32, then bf16 hi/lo split ----
            # hidT [h, b_loc] -> SBUF [128, hc_n, b_loc] (k-chunk c holds rows
            # c*128 + p); W [h, h] -> SBUF [128, hc_n, h] the same way.
            hidT_sb = consts.tile([P, hc_n, b_loc], F32)
            nc.sync.dma_start(
                out=hidT_sb, in_=hidT.rearrange("(c p) b -> p c b", p=P)
            )
            w_sb = consts.tile([P, hc_n, h], F32)
            nc.sync.dma_start(out=w_sb, in_=w.rearrange("(c p) j -> p c j", p=P))

            # v [b_loc, h] in PSUM: stationary = hidT chunk (tiny), moving = W
            vps = psp.tile([b_loc, h], F32, tag="ps")
            for kc in range(hc_n):
                for j0 in range(0, h, SCHUNK):
                    j1 = min(j0 + SCHUNK, h)
                    nc.tensor.matmul(
                        vps[:, j0:j1],
                        hidT_sb[:, kc, :],
                        w_sb[:, kc, j0:j1],
                        start=(kc == 0),
                        stop=(kc == hc_n - 1),
                    )
            v_sb = consts.tile([b_loc, h], F32)
            nc.vector.tensor_copy(v_sb, vps)

            # transpose [b_loc, 128] chunks -> vT [128, hc_n*b_loc] via PE
            ident = consts.tile([b_loc, b_loc], F32)
            make_identity(nc, ident)
            vT = consts.tile([P, hc_n * b_loc], F32)
            for hcc in range(hc_n):
                tp = psp.tile([P, b_loc], F32, tag="ps")
                nc.tensor.transpose(
                    tp, v_sb[:, hcc * P : (hcc + 1) * P], ident
                )
                nc.vector.tensor_copy(vT[:, hcc * b_loc : (hcc + 1) * b_loc], tp)

            # bf16 hi/lo split of vT (vT = vhi + vlo + O(2^-18))
            vhi = consts.tile([P, hc_n * b_loc], BF16)
            nc.vector.tensor_copy(vhi, vT)
            vhi_f = consts.tile([P, hc_n * b_loc], F32)
            nc.vector.tensor_copy(vhi_f, vhi)
            vres = consts.tile([P, hc_n * b_loc], F32)
            nc.vector.tensor_sub(vres, vT, vhi_f)
            vlo = consts.tile([P, hc_n * b_loc], BF16)
            nc.vector.tensor_copy(vlo, vres)

            # ---- phase 2: e[b, s] = vT[:, b] . enc[b, :, s], then softmax ----
            for bl in range(b_loc):
                # per-b enc tiles [128, hc_n, s] bf16, DMA'd in halves so the
                # PE can start on the first half while the second streams
                et = {}
                for nm, src in (("ehi", ehi), ("elo", elo)):
                    t = encp.tile([P, hc_n, s], BF16, tag=nm)
                    for ch in range(0, hc_n, hc_half):
                        nc.sync.dma_start(
                            out=t[:, ch : ch + hc_half, :],
                            in_=src[bl, ch * P : (ch + hc_half) * P, :].rearrange(
                                "(c p) s -> p c s", p=P
                            ),
                        )
                    et[nm] = t

                eps = psp.tile([1, s], F32, tag="ps")
                # e = vhi.ehi + vhi.elo + vlo.ehi, accumulated over h-chunks;
                # term order per h-chunk keeps LDWEIGHTS switches to 2 per hc
                terms = [(vhi, "ehi"), (vhi, "elo"), (vlo, "ehi")]
                for hcc in range(hc_n):
                    for ti, (vv, enm) in enumerate(terms):
                        lhs = vv[:, hcc * b_loc + bl : hcc * b_loc + bl + 1]
                        for sc in range(sc_n):
                            nc.tensor.matmul(
                                eps[0:1, sc * SCHUNK : (sc + 1) * SCHUNK],
                                lhs,
                                et[enm][:, hcc, sc * SCHUNK : (sc + 1) * SCHUNK],
                                start=(hcc == 0 and ti == 0),
                                stop=(hcc == hc_n - 1 and ti == len(terms) - 1),
                            )

                # softmax over s (single partition lane)
                negmax = small.tile([1, 1], F32, tag="negmax")
                nc.vector.tensor_reduce(
                    negmax,
                    eps[0:1, :],
                    axis=mybir.AxisListType.X,
                    op=mybir.AluOpType.max,
                    negate=True,
                )
                psb = small.tile([1, s], F32, tag="p")
                ssum = small.tile([1, 1], F32, tag="ssum")
                nc.scalar.activation(
                    psb,
                    eps[0:1, :],
                    mybir.ActivationFunctionType.Exp,
                    bias=negmax,
                    scale=1.0,
                    accum_out=ssum,
                )
                rinv = small.tile([1, 1], F32, tag="rinv")
                nc.vector.reciprocal(rinv, ssum)
                osb = small.tile([1, s], F32, tag="o")
                nc.vector.tensor_scalar_mul(osb, psb, rinv)
                nc.sync.dma_start(out=out[bl : bl + 1, :], in_=osb)

    nc.compile()
    return nc


def _make_in_maps(hidden, encoder_outputs, W_attn):
    hidden = np.ascontiguousarray(np.asarray(hidden, dtype=np.float32))
    enc = np.asarray(encoder_outputs, dtype=np.float32)
    W = np.ascontiguousarray(np.asarray(W_attn, dtype=np.float32))

    # [S, B, H] -> [B, H, S] relayout + bf16 hi/lo split (pure data movement
    # plus rounding; same byte count as the f32 original)
    encT = np.ascontiguousarray(enc.transpose(1, 2, 0))
    ehi = encT.astype(BF16_NP)
    elo = (encT - ehi.astype(np.float32)).astype(BF16_NP)
    hidT_full = np.ascontiguousarray(hidden[0].T)  # [H, B]

    in_maps = []
    for i in range(N_CORES):
        lo, hi = i * B_LOC, (i + 1) * B_LOC
        in_maps.append(
            {
                "ehi": np.ascontiguousarray(ehi[lo:hi]),
                "elo": np.ascontiguousarray(elo[lo:hi]),
                "hidT": np.ascontiguousarray(hidT_full[:, lo:hi]),
                "w": W,
            }
        )
    return in_maps


def run_spmd(hidden, encoder_outputs, W_attn, b_attn=None, trace=False):
    """Run on all 8 cores; returns (out [B,1,S], BassKernelResults)."""
    in_maps = _make_in_maps(hidden, encoder_outputs, W_attn)
    nc = build_program()
    res = run_bass_kernel_spmd(nc, in_maps, list(range(N_CORES)), trace=trace)
    out = np.concatenate([r["out"] for r in res.results], axis=0)  # [B, S]
    return np.ascontiguousarray(out[:, None, :].astype(np.float32)), res


def kernel(hidden, encoder_outputs, W_attn, b_attn):
    # b_attn contributes a per-b constant to the energies; softmax over s is
    # invariant to it, so it is (exactly) unused.
    out, _ = run_spmd(hidden, encoder_outputs, W_attn, b_attn)
    return out
